# revision 12
# baseline (speedup 1.0000x reference)
"""MicroMamba Trainium2 kernel: 8-core SPMD (2 batch groups x 4-way d_inner shard).

Self-contained: hardcodes shapes/sharding from the problem spec.
kernel(**inputs) takes full unsharded inputs, returns full logits [2,2048,32000].

Sharding: cores 0-3 run batch 0, cores 4-7 batch 1. Within a batch group,
core q owns d_inner channels [384q, 384q+384). Activations live in
[feature, time] (transposed) layout so the depthwise conv and the selective
scan run along the free dimension. The scan uses the VectorE
tensor_tensor_scan instruction over a [(channel,state), time] layout; the
16x state broadcast of dt/u is done with selector matmuls on TensorE, and
the y readout (sum over state) with selector matmuls accumulating in PSUM.
x_proj and out_proj contract over the full d_inner, so their partials are
AllReduced within each batch group. The LM head is sharded 4-way over vocab
within each batch group (each core computes [8000, 2048] logits^T).
"""
import numpy as np
import ml_dtypes

import concourse.bass as bass
import concourse.tile as tile
from concourse import mybir
from concourse.bass_utils import run_bass_kernel_spmd
from concourse.vector_clock import ScopedClock

# ---------------------------------------------------------------- tile fix --
# This walrus build rejects >1 sem-wait on some instruction encodings (CTRL
# drain/nop, S2S2D2 scan). After Tile scheduling, spill excess waits onto
# same-engine nops inserted right before the overloaded instruction.
MAX_WAITS = 1


def _redistribute_waits(nc, cap=MAX_WAITS):
    work = []
    for f in nc.m.functions:
        for bb in f.blocks:
            insts = bb.instructions
            ov = set()
            for i, inst in enumerate(insts):
                si = inst.sync_info
                if si is not None and si.on_wait and len(si.on_wait) > cap:
                    ov.add(i)
            if ov:
                work.append((bb, insts, ov))
    if not work:
        return
    curbb = nc.cur_bb.bb

    def make_nop(engine, waits):
        eng = nc.engines[engine]
        nop = eng.nop(nofuse=True).ins
        lst = curbb.instructions
        assert lst[-1].name == nop.name, "nop did not append to cur_bb tail"
        curbb.instructions = lst[:-1]
        nop.sync_info = mybir.SyncInfo(on_wait=list(waits), on_update=[])
        return nop

    for bb, insts, ov in work:
        newlist = []
        for i, inst in enumerate(insts):
            if i in ov:
                si = inst.sync_info
                waits = list(si.on_wait)
                si.on_wait = waits[:cap]
                rest = waits[cap:]
                while rest:
                    chunk, rest = rest[:cap], rest[cap:]
                    newlist.append(make_nop(inst.engine, chunk))
            newlist.append(inst)
        bb.instructions = newlist


class _TileCtx(tile.TileContext):
    def _drain_and_barrier(self, tick_clock, wait_clock):
        nc = self.nc
        drain_inst = nc.sync.drain()
        wait_clock.add_sem_waits(
            drain_inst.ins, ScopedClock({None: tick_clock.global_clock})
        )
        si = drain_inst.ins.sync_info
        if si is not None and si.on_wait and len(si.on_wait) > MAX_WAITS:
            waits = list(si.on_wait)
            si.on_wait = waits[:MAX_WAITS]
            rest = waits[MAX_WAITS:]
            while rest:
                extra = nc.sync.drain()
                chunk, rest = rest[:MAX_WAITS], rest[MAX_WAITS:]
                esi = extra.ins.sync_info
                if esi is None:
                    extra.ins.sync_info = mybir.SyncInfo(
                        on_wait=list(chunk), on_update=[]
                    )
                else:
                    esi.on_wait = list(chunk)
        nc.all_engine_barrier()
        assert self.sems is not None
        popped = nc._tile_sem_poison_stack.pop()
        assert popped is self._sem_poison
        nc.clear_and_free_semaphores(list(self.sems.allocated().values()))
        nc.all_engine_barrier()

    def __exit__(self, exc_type, exc, tb):
        ret = super().__exit__(exc_type, exc, tb)
        if exc_type is None:
            _redistribute_waits(self.nc)
        return ret


# ---------------------------------------------------------------- constants --
VOCAB, DIM, DEPTH = 32000, 768, 4
DS, DCONV, DIN, DTR = 16, 4, 1536, 48
B, T = 2, 2048
NCORES = 8
DSH = DIN // 4          # 384 channels per core
VSH = VOCAB // 4        # 8000 vocab rows per core (within its batch group)
NCT = DSH // 128        # 3 channel tiles
NKT = DIM // 128        # 6 dim tiles
NJ = T // 512           # 4 time chunks of 512
NG = DSH * DS // 128    # 48 (c,s) tiles
EPS = 1e-5

FP32 = mybir.dt.float32
BF16 = mybir.dt.bfloat16
AF = mybir.ActivationFunctionType
OP = mybir.AluOpType
BF = ml_dtypes.bfloat16

REPLICA_GROUPS = [[0, 1, 2, 3], [4, 5, 6, 7]]


def _build_program():
    nc = bass.Bass()

    # ---- I/O -------------------------------------------------------------
    d_xT = nc.dram_tensor("xT", [DIM, T], BF16, kind="ExternalInput")
    d_w_in = nc.dram_tensor("w_in_T", [DEPTH, DIM, 2 * DSH], BF16, kind="ExternalInput")
    d_conv_w = nc.dram_tensor("conv_wk", [DEPTH, DCONV, NCT, 128], FP32, kind="ExternalInput")
    d_conv_b = nc.dram_tensor("conv_b", [DEPTH, NCT, 128], FP32, kind="ExternalInput")
    d_w_x = nc.dram_tensor("w_x_T", [DEPTH, DSH, 80], BF16, kind="ExternalInput")
    d_w_dt = nc.dram_tensor("w_dt_T", [DEPTH, DTR, DSH], BF16, kind="ExternalInput")
    d_b_dt = nc.dram_tensor("b_dt", [DEPTH, NCT, 128], FP32, kind="ExternalInput")
    d_a_cs = nc.dram_tensor("a_cs", [DEPTH, NG, 128], FP32, kind="ExternalInput")
    d_dp = nc.dram_tensor("dp", [DEPTH, NCT, 128], FP32, kind="ExternalInput")
    d_w_out = nc.dram_tensor("w_out_T", [DEPTH, DSH, DIM], BF16, kind="ExternalInput")
    d_ln_g = nc.dram_tensor("ln_g", [NKT, 128], FP32, kind="ExternalInput")
    d_ln_b = nc.dram_tensor("ln_b", [NKT, 128], FP32, kind="ExternalInput")
    d_w_head = nc.dram_tensor("w_head_T", [DIM, VSH], BF16, kind="ExternalInput")
    d_sel8g = nc.dram_tensor("sel8g", [16, 128, 128], BF16, kind="ExternalInput")
    d_selg = nc.dram_tensor("selg", [16, 128, 128], BF16, kind="ExternalInput")
    d_sel16bc = nc.dram_tensor("sel16bc", [16, 128], BF16, kind="ExternalInput")
    d_ones_row = nc.dram_tensor("ones_row", [1, 128], FP32, kind="ExternalInput")
    d_ones_col = nc.dram_tensor("ones_col", [128, 1], FP32, kind="ExternalInput")
    d_ones_colb = nc.dram_tensor("ones_colb", [128, 1], BF16, kind="ExternalInput")

    d_logT = nc.dram_tensor("logitsT", [VSH, T], FP32, kind="ExternalOutput")

    with _TileCtx(nc) as tc:
        with (
            tc.tile_pool(name="const", bufs=1) as cpool,
            tc.tile_pool(name="xcur", bufs=1) as xpool,
            tc.tile_pool(name="wts", bufs=1) as wpool,
            tc.tile_pool(name="whp", bufs=2) as whpool,
            tc.tile_pool(name="act", bufs=1) as apool,
            tc.tile_pool(name="big", bufs=5) as bpool,
            tc.tile_pool(name="scan", bufs=2) as spool,
            tc.tile_pool(name="tmp", bufs=2) as tpool,
            tc.tile_pool(name="ln", bufs=1) as lpool,
            tc.tile_pool(name="ps", bufs=2, space="PSUM") as pspool,
            tc.tile_pool(name="psy", bufs=1, space="PSUM") as pypool,
            tc.tile_pool(name="dram", bufs=2, space="DRAM") as dpool,
        ):
            # ---- constants ---------------------------------------------
            sel8g = cpool.tile([128, 16 * 128], BF16, tag="sel8g")
            selg = cpool.tile([128, 16 * 128], BF16, tag="selg")
            sel16bc = cpool.tile([16, 128], BF16, tag="sel16bc")
            ones_row = cpool.tile([1, 128], FP32, tag="ones_row")
            ones_col = cpool.tile([128, 1], FP32, tag="ones_col")
            ones_colb = cpool.tile([128, 1], BF16, tag="ones_colb")
            for gg in range(16):
                nc.sync.dma_start(sel8g[:, 128 * gg:128 * (gg + 1)], d_sel8g[gg])
                nc.sync.dma_start(selg[:, 128 * gg:128 * (gg + 1)], d_selg[gg])
            nc.sync.dma_start(sel16bc[:], d_sel16bc[:])
            nc.sync.dma_start(ones_row[:], d_ones_row[:])
            nc.sync.dma_start(ones_col[:], d_ones_col[:])
            nc.sync.dma_start(ones_colb[:], d_ones_colb[:])
            lng = cpool.tile([128, NKT], FP32, tag="lng")
            lnb = cpool.tile([128, NKT], FP32, tag="lnb")
            nc.sync.dma_start(lng[:], d_ln_g.rearrange("k p -> p k"))
            nc.sync.dma_start(lnb[:], d_ln_b.rearrange("k p -> p k"))

            # ---- x current (bf16, [128, T] x 6) ------------------------
            xc = []
            for k in range(NKT):
                t_ = xpool.tile([128, T], BF16, tag=f"xc{k}", name=f"xc{k}")
                nc.sync.dma_start(t_[:], d_xT[128 * k:128 * (k + 1), :])
                xc.append(t_)

            for l in range(DEPTH):
                # ---- load layer weights --------------------------------
                w_in = []
                for k in range(NKT):
                    t_ = wpool.tile([128, 2 * DSH], BF16, tag=f"win{k}",
                                    name=f"win{k}")
                    nc.sync.dma_start(t_[:], d_w_in[l, 128 * k:128 * (k + 1), :])
                    w_in.append(t_)
                w_x = []
                for c in range(NCT):
                    t_ = wpool.tile([128, 80], BF16, tag=f"wx{c}", name=f"wx{c}")
                    nc.sync.dma_start(t_[:], d_w_x[l, 128 * c:128 * (c + 1), :])
                    w_x.append(t_)
                w_dt = wpool.tile([DTR, DSH], BF16, tag="wdt", name="wdt")
                nc.sync.dma_start(w_dt[:], d_w_dt[l])
                w_out = []
                for c in range(NCT):
                    t_ = wpool.tile([128, DIM], BF16, tag=f"wo{c}", name=f"wo{c}")
                    nc.sync.dma_start(t_[:], d_w_out[l, 128 * c:128 * (c + 1), :])
                    w_out.append(t_)
                convw = wpool.tile([128, DCONV * NCT], FP32, tag="convw",
                                   name="convw")
                nc.sync.dma_start(convw[:], d_conv_w[l].rearrange("k c p -> p (k c)"))
                convb = wpool.tile([128, NCT], FP32, tag="convb", name="convb")
                nc.sync.dma_start(convb[:], d_conv_b[l].rearrange("c p -> p c"))
                bdt = wpool.tile([128, NCT], FP32, tag="bdt", name="bdt")
                nc.sync.dma_start(bdt[:], d_b_dt[l].rearrange("c p -> p c"))
                acs = wpool.tile([128, NG], FP32, tag="acs", name="acs")
                nc.sync.dma_start(acs[:], d_a_cs[l].rearrange("g p -> p g"))
                dpw = wpool.tile([128, NCT], FP32, tag="dpw", name="dpw")
                nc.sync.dma_start(dpw[:], d_dp[l].rearrange("c p -> p c"))

                # ---- in_proj: xz^T [768out, T]; x_in has 4-col halo ----
                _sc_inproj = nc.named_scope(f"L{l}_inproj"); _sc_inproj.__enter__()
                x_in = []
                z_silu = []
                for m in range(6):
                    if m < 3:
                        xt = bpool.tile([128, T + 4], BF16, tag="big",
                                        name=f"xin{m}")
                        nc.vector.memset(xt[:, 0:4], 0.0)
                        x_in.append(xt)
                    else:
                        zt = apool.tile([128, T], BF16, tag=f"zs{m - 3}",
                                        name=f"zs{m - 3}")
                        z_silu.append(zt)
                    for j in range(NJ):
                        ps = pspool.tile([128, 512], FP32, tag="ps_a", name="psa")
                        for k in range(NKT):
                            nc.tensor.matmul(
                                ps[:], w_in[k][:, 128 * m:128 * (m + 1)],
                                xc[k][:, 512 * j:512 * (j + 1)],
                                start=(k == 0), stop=(k == NKT - 1))
                        if m < 3:
                            nc.scalar.copy(
                                x_in[m][:, 4 + 512 * j:4 + 512 * (j + 1)], ps[:])
                        else:
                            nc.scalar.activation(
                                z_silu[m - 3][:, 512 * j:512 * (j + 1)], ps[:],
                                AF.Silu)

                _sc_inproj.__exit__(None, None, None)
                _sc_conv = nc.named_scope(f"L{l}_conv"); _sc_conv.__enter__()
                # ---- conv + silu -> x_ssm ------------------------------
                x_ssm = []
                for c in range(NCT):
                    cv = tpool.tile([128, T], BF16, tag="cv", name="cv")
                    nc.vector.tensor_scalar(
                        cv[:], x_in[c][:, 1:1 + T],
                        convw[:, 0 * NCT + c:0 * NCT + c + 1], None, OP.mult)
                    for k in range(1, DCONV):
                        cv2 = tpool.tile([128, T], BF16, tag="cv", name="cv")
                        nc.vector.scalar_tensor_tensor(
                            cv2[:], x_in[c][:, 1 + k:1 + k + T],
                            convw[:, k * NCT + c:k * NCT + c + 1], cv[:],
                            OP.mult, OP.add)
                        cv = cv2
                    xs = apool.tile([128, T], BF16, tag=f"xssm{c}",
                                    name=f"xssm{c}")
                    nc.scalar.activation(xs[:], cv[:], AF.Silu,
                                         bias=convb[:, c:c + 1])
                    x_ssm.append(xs)

                _sc_conv.__exit__(None, None, None)
                _sc_xproj = nc.named_scope(f"L{l}_xproj"); _sc_xproj.__enter__()
                # ---- x_proj (partial) + AllReduce ----------------------
                dblp = tpool.tile([80, T], BF16, tag="dblp", name="dblp",
                                  bufs=1)
                for j in range(NJ):
                    ps = pspool.tile([80, 512], FP32, tag="ps_a", name="psa")
                    for c in range(NCT):
                        nc.tensor.matmul(
                            ps[:], w_x[c][:],
                            x_ssm[c][:, 512 * j:512 * (j + 1)],
                            start=(c == 0), stop=(c == NCT - 1))
                    nc.scalar.copy(dblp[:, 512 * j:512 * (j + 1)], ps[:])
                d_dbl_p = dpool.tile([80, T], FP32, tag="dbl_p", name="dbl_p")
                d_dbl_f = dpool.tile([80, T], FP32, tag="dbl_f", name="dbl_f")
                nc.gpsimd.dma_start(d_dbl_p[:], dblp[:])
                nc.gpsimd.collective_compute(
                    "AllReduce", OP.add, replica_groups=REPLICA_GROUPS,
                    ins=[d_dbl_p.opt()], outs=[d_dbl_f.opt()])
                # split into base-0 tiles (matmul operands must share base 0)
                dbl_dt = apool.tile([DTR, T], BF16, tag="dbl_dt", name="dbl_dt")
                dbl_B = apool.tile([DS, T], BF16, tag="dbl_B", name="dbl_B")
                dbl_C = apool.tile([DS, T], BF16, tag="dbl_C", name="dbl_C")
                nc.gpsimd.dma_start(dbl_dt[:], d_dbl_f[0:DTR, :])
                nc.gpsimd.dma_start(dbl_B[:], d_dbl_f[DTR:DTR + DS, :])
                nc.gpsimd.dma_start(dbl_C[:], d_dbl_f[DTR + DS:80, :])

                _sc_xproj.__exit__(None, None, None)
                _sc_bc = nc.named_scope(f"L{l}_bc"); _sc_bc.__enter__()
                # ---- B_bc / C_bc [128, T] (bf16) -----------------------
                Bbc = apool.tile([128, T], BF16, tag="Bbc", name="Bbc")
                Cbc = apool.tile([128, T], BF16, tag="Cbc", name="Cbc")
                for j in range(NJ):
                    psb = pspool.tile([128, 512], FP32, tag="ps_a", name="psa")
                    nc.tensor.matmul(psb[:], sel16bc[:],
                                     dbl_B[:, 512 * j:512 * (j + 1)],
                                     start=True, stop=True)
                    nc.scalar.copy(Bbc[:, 512 * j:512 * (j + 1)], psb[:])
                    psc = pspool.tile([128, 512], FP32, tag="ps_a", name="psa")
                    nc.tensor.matmul(psc[:], sel16bc[:],
                                     dbl_C[:, 512 * j:512 * (j + 1)],
                                     start=True, stop=True)
                    nc.scalar.copy(Cbc[:, 512 * j:512 * (j + 1)], psc[:])

                _sc_bc.__exit__(None, None, None)
                _sc_scan = nc.named_scope(f"L{l}_scan"); _sc_scan.__enter__()
                # ---- scan (dt/u computed per channel-tile) -------------
                ydp = []
                for c in range(NCT):
                    dt_ = apool.tile([128, T], BF16, tag="dt", name="dt")
                    for j in range(NJ):
                        ps = pspool.tile([128, 512], FP32, tag="ps_a", name="psa")
                        nc.tensor.matmul(
                            ps[:], w_dt[:, 128 * c:128 * (c + 1)],
                            dbl_dt[:, 512 * j:512 * (j + 1)],
                            start=True, stop=True)
                        # softplus(x) = ln(1 + exp(x)); Softplus has no ACT table
                        spe = tpool.tile([128, 512], FP32, tag="spe", name="spe")
                        nc.scalar.activation(spe[:], ps[:], AF.Exp,
                                             bias=bdt[:, c:c + 1])
                        sp1 = tpool.tile([128, 512], FP32, tag="sp1", name="sp1")
                        nc.vector.tensor_scalar(sp1[:], spe[:], 1.0, None, OP.add)
                        nc.scalar.activation(
                            dt_[:, 512 * j:512 * (j + 1)], sp1[:], AF.Ln)
                    u_ = apool.tile([128, T], BF16, tag="u", name="u")
                    nc.vector.tensor_mul(u_[:], dt_[:], x_ssm[c][:])

                    yps = [pypool.tile([128, 512], FP32, tag=f"yps{j}",
                                       name=f"yps{j}") for j in range(NJ)]
                    for gg in range(16):
                        g = 16 * c + gg
                        dA = spool.tile([128, T], BF16, tag="scan_a", name="dA")
                        for j in range(NJ):
                            ps = pspool.tile([128, 512], FP32, tag="ps_c",
                                             name="psc")
                            nc.tensor.matmul(
                                ps[:], sel8g[:, 128 * gg:128 * (gg + 1)],
                                dt_[:, 512 * j:512 * (j + 1)],
                                start=True, stop=True)
                            nc.scalar.activation(
                                dA[:, 512 * j:512 * (j + 1)], ps[:], AF.Exp,
                                scale=acs[:, g:g + 1])
                        uB = spool.tile([128, T], BF16, tag="scan_b", name="uB")
                        for j in range(NJ):
                            ps = pspool.tile([128, 512], FP32, tag="ps_c",
                                             name="psc")
                            nc.tensor.matmul(
                                ps[:], sel8g[:, 128 * gg:128 * (gg + 1)],
                                u_[:, 512 * j:512 * (j + 1)],
                                start=True, stop=True)
                            nc.vector.tensor_mul(
                                uB[:, 512 * j:512 * (j + 1)], ps[:],
                                Bbc[:, 512 * j:512 * (j + 1)])
                        h = spool.tile([128, T], BF16, tag="scan_a", name="h")
                        nc.vector.tensor_tensor_scan(
                            h[:], dA[:], uB[:], 0.0, OP.mult, OP.add)
                        hC = spool.tile([128, T], BF16, tag="scan_b", name="hC")
                        nc.vector.tensor_mul(hC[:], h[:], Cbc[:])
                        for j in range(NJ):
                            nc.tensor.matmul(
                                yps[j][:], selg[:, 128 * gg:128 * (gg + 1)],
                                hC[:, 512 * j:512 * (j + 1)],
                                start=(gg == 0), stop=(gg == 15))
                    # y + x_ssm*Dp directly from PSUM
                    yd = bpool.tile([128, T], BF16, tag="big", name=f"ydp{c}")
                    for j in range(NJ):
                        nc.vector.scalar_tensor_tensor(
                            yd[:, 512 * j:512 * (j + 1)],
                            x_ssm[c][:, 512 * j:512 * (j + 1)],
                            dpw[:, c:c + 1], yps[j][:], OP.mult, OP.add)
                    ydp.append(yd)

                _sc_scan.__exit__(None, None, None)
                _sc_out = nc.named_scope(f"L{l}_outproj"); _sc_out.__enter__()
                # ---- gate + out_proj -----------------------------------
                yg = []
                for c in range(NCT):
                    t2 = bpool.tile([128, T], BF16, tag="big", name=f"yg{c}")
                    nc.vector.tensor_mul(t2[:], ydp[c][:], z_silu[c][:])
                    yg.append(t2)

                d_xn_p = dpool.tile([DIM, T], FP32, tag="xn_p", name="xn_p")
                d_xn_f = dpool.tile([DIM, T], FP32, tag="xn_f", name="xn_f")
                for m in range(NKT):
                    xnp = tpool.tile([128, T], FP32, tag="xnp", name="xnp")
                    for j in range(NJ):
                        ps = pspool.tile([128, 512], FP32, tag="ps_a", name="psa")
                        for c in range(NCT):
                            nc.tensor.matmul(
                                ps[:], w_out[c][:, 128 * m:128 * (m + 1)],
                                yg[c][:, 512 * j:512 * (j + 1)],
                                start=(c == 0), stop=(c == NCT - 1))
                        nc.scalar.copy(xnp[:, 512 * j:512 * (j + 1)], ps[:])
                    nc.sync.dma_start(d_xn_p[128 * m:128 * (m + 1), :], xnp[:])
                nc.gpsimd.collective_compute(
                    "AllReduce", OP.add, replica_groups=REPLICA_GROUPS,
                    ins=[d_xn_p.opt()], outs=[d_xn_f.opt()])
                for k in range(NKT):
                    nc.gpsimd.dma_start(xc[k][:], d_xn_f[128 * k:128 * (k + 1), :])
                _sc_out.__exit__(None, None, None)

            # ---- layernorm (per 512-chunk, in-place on xc) -------------
            _sc_ln = nc.named_scope("LN"); _sc_ln.__enter__()
            for j in range(NJ):
                js = slice(512 * j, 512 * (j + 1))
                ps1 = pspool.tile([1, 512], FP32, tag="ps_a", name="psa")
                for k in range(NKT):
                    nc.tensor.matmul(ps1[:], ones_colb[:], xc[k][:, js],
                                     start=(k == 0), stop=(k == NKT - 1))
                ps2 = pspool.tile([1, 512], FP32, tag="ps_a", name="psa")
                for k in range(NKT):
                    sqt = tpool.tile([128, 512], FP32, tag="sq", name="sq")
                    nc.scalar.activation(sqt[:], xc[k][:, js], AF.Square)
                    nc.tensor.matmul(ps2[:], ones_col[:], sqt[:],
                                     start=(k == 0), stop=(k == NKT - 1))
                mu = lpool.tile([1, 512], FP32, tag="mu", name="mu")
                nc.scalar.mul(mu[:], ps1[:], 1.0 / DIM)
                musq = lpool.tile([1, 512], FP32, tag="musq", name="musq")
                nc.scalar.activation(musq[:], mu[:], AF.Square)
                s2n = lpool.tile([1, 512], FP32, tag="s2n", name="s2n")
                nc.scalar.mul(s2n[:], ps2[:], 1.0 / DIM)
                var = lpool.tile([1, 512], FP32, tag="var", name="var")
                nc.vector.tensor_sub(var[:], s2n[:], musq[:])
                vare = lpool.tile([1, 512], FP32, tag="vare", name="vare")
                nc.vector.tensor_scalar(vare[:], var[:], float(EPS), None, OP.add)
                std = lpool.tile([1, 512], FP32, tag="std", name="std")
                nc.scalar.activation(std[:], vare[:], AF.Sqrt)
                rstd = lpool.tile([1, 512], FP32, tag="rstd", name="rstd")
                nc.vector.reciprocal(rstd[:], std[:])
                # broadcast to 128 partitions
                psm = pspool.tile([128, 512], FP32, tag="ps_a", name="psa")
                nc.tensor.matmul(psm[:], ones_row[:], mu[:], start=True, stop=True)
                mub = lpool.tile([128, 512], BF16, tag="mub", name="mub")
                nc.scalar.copy(mub[:], psm[:])
                psr = pspool.tile([128, 512], FP32, tag="ps_a", name="psa")
                nc.tensor.matmul(psr[:], ones_row[:], rstd[:], start=True,
                                 stop=True)
                rsb = lpool.tile([128, 512], BF16, tag="rsb", name="rsb")
                nc.scalar.copy(rsb[:], psr[:])
                for k in range(NKT):
                    d1 = lpool.tile([128, 512], BF16, tag="lnd1", name="lnd1")
                    nc.vector.tensor_sub(d1[:], xc[k][:, js], mub[:])
                    d2 = lpool.tile([128, 512], BF16, tag="lnd2", name="lnd2")
                    nc.vector.tensor_mul(d2[:], d1[:], rsb[:])
                    nc.scalar.activation(xc[k][:, js], d2[:], AF.Identity,
                                         bias=lnb[:, k:k + 1],
                                         scale=lng[:, k:k + 1])

            _sc_ln.__exit__(None, None, None)
            _sc_head = nc.named_scope("head"); _sc_head.__enter__()
            # ---- head: logits^T [8000, 2048] ---------------------------
            vt_sizes = [128] * (VSH // 128) + ([VSH % 128] if VSH % 128 else [])
            for vi, vsz in enumerate(vt_sizes):
                wh = whpool.tile([128, NKT * 128], BF16, tag="wh", name="wh")
                for k in range(NKT):
                    nc.sync.dma_start(
                        wh[:, 128 * k:128 * k + vsz],
                        d_w_head[128 * k:128 * (k + 1), 128 * vi:128 * vi + vsz])
                for j in range(NJ):
                    ps = pspool.tile([128, 512], FP32, tag="ps_a", name="psa")
                    for k in range(NKT):
                        nc.tensor.matmul(
                            ps[0:vsz, :], wh[:, 128 * k:128 * k + vsz],
                            xc[k][:, 512 * j:512 * (j + 1)],
                            start=(k == 0), stop=(k == NKT - 1))
                    lt = tpool.tile([128, 512], FP32, tag="lt", name="lt",
                                     bufs=1)
                    nc.scalar.copy(lt[0:vsz, :], ps[0:vsz, :])
                    nc.sync.dma_start(
                        d_logT[128 * vi:128 * vi + vsz, 512 * j:512 * (j + 1)],
                        lt[0:vsz, :])
            _sc_head.__exit__(None, None, None)
    return nc


# ------------------------------------------------------------------ host ----
_PROG = None


def _get_program():
    global _PROG
    if _PROG is None:
        _PROG = _build_program()
    return _PROG


def _selectors():
    p_ar = np.arange(128)
    sel8g = np.zeros((16, 128, 128), BF)
    selg = np.zeros((16, 128, 128), BF)
    for gg in range(16):
        sel8g[gg, 8 * gg + p_ar // 16, p_ar] = 1.0
        selg[gg, p_ar, 8 * gg + p_ar // 16] = 1.0
    sel16bc = np.zeros((16, 128), BF)
    sel16bc[p_ar % 16, p_ar] = 1.0
    return sel8g, selg, sel16bc


def _prep_core_inputs(core, ids, emb, W_in, conv_w, conv_b, W_x, W_dt, b_dt,
                      A_log, Dp, W_out, ln_g, ln_b, W_head, sels):
    b = core // 4
    q = core % 4
    csl = slice(q * DSH, (q + 1) * DSH)
    vsl = slice(q * VSH, (q + 1) * VSH)
    sel8g, selg, sel16bc = sels

    xT = np.ascontiguousarray(emb[ids[b]].T).astype(BF)   # [768, 2048]

    w_in_T = np.empty((DEPTH, DIM, 2 * DSH), BF)
    w_x_T = np.empty((DEPTH, DSH, 80), BF)
    w_dt_T = np.empty((DEPTH, DTR, DSH), BF)
    w_out_T = np.empty((DEPTH, DSH, DIM), BF)
    conv_wk = np.empty((DEPTH, DCONV, NCT, 128), np.float32)
    conv_b_r = np.empty((DEPTH, NCT, 128), np.float32)
    b_dt_r = np.empty((DEPTH, NCT, 128), np.float32)
    dp_r = np.empty((DEPTH, NCT, 128), np.float32)
    a_cs = np.empty((DEPTH, NG, 128), np.float32)
    p_ar = np.arange(128)
    for l in range(DEPTH):
        rows = np.concatenate([np.arange(csl.start, csl.stop),
                               DIN + np.arange(csl.start, csl.stop)])
        w_in_T[l] = W_in[l][rows, :].T.astype(BF)
        w_x_T[l] = W_x[l][:, csl].T.astype(BF)
        w_dt_T[l] = W_dt[l][csl, :].T.astype(BF)
        w_out_T[l] = W_out[l][:, csl].T.astype(BF)
        conv_wk[l] = conv_w[l][csl, :].T.reshape(DCONV, NCT, 128)
        conv_b_r[l] = conv_b[l][csl].reshape(NCT, 128)
        b_dt_r[l] = b_dt[l][csl].reshape(NCT, 128)
        dp_r[l] = Dp[l][csl].reshape(NCT, 128)
        A = -np.exp(A_log[l][csl, :])                     # [384, 16]
        for g in range(NG):
            a_cs[l, g] = A[8 * g + p_ar // 16, p_ar % 16]

    return {
        "xT": xT,
        "w_in_T": w_in_T, "conv_wk": conv_wk, "conv_b": conv_b_r,
        "w_x_T": w_x_T, "w_dt_T": w_dt_T, "b_dt": b_dt_r,
        "a_cs": a_cs, "dp": dp_r, "w_out_T": w_out_T,
        "ln_g": np.ascontiguousarray(ln_g.reshape(NKT, 128)).astype(np.float32),
        "ln_b": np.ascontiguousarray(ln_b.reshape(NKT, 128)).astype(np.float32),
        "w_head_T": np.ascontiguousarray(W_head[vsl, :].T).astype(BF),
        "sel8g": sel8g, "selg": selg, "sel16bc": sel16bc,
        "ones_row": np.ones((1, 128), np.float32),
        "ones_col": np.ones((128, 1), np.float32),
        "ones_colb": np.ones((128, 1), BF),
    }


def kernel(input_ids, emb, W_in, conv_w, conv_b, W_x, W_dt, b_dt, A_log, Dp,
           W_out, ln_g, ln_b, W_head):
    ids = np.asarray(input_ids).astype(np.int64)
    args = [np.asarray(a, np.float32) for a in
            (emb, W_in, conv_w, conv_b, W_x, W_dt, b_dt, A_log, Dp, W_out,
             ln_g, ln_b, W_head)]
    nc = _get_program()
    sels = _selectors()
    in_maps = [_prep_core_inputs(c, ids, *args, sels) for c in range(NCORES)]
    res = run_bass_kernel_spmd(nc, in_maps, list(range(NCORES))).results
    out = np.empty((B, T, VOCAB), np.float32)
    for b in range(B):
        for q in range(4):
            out[b, :, q * VSH:(q + 1) * VSH] = res[4 * b + q]["logitsT"].T
    return out


# revision 13
# speedup vs baseline: 1.0387x; 1.0387x over previous
"""MicroMamba Trainium2 kernel: 8-core SPMD (2 batch groups x 4-way d_inner shard).

Self-contained: hardcodes shapes/sharding from the problem spec.
kernel(**inputs) takes full unsharded inputs, returns full logits [2,2048,32000].

Sharding: cores 0-3 run batch 0, cores 4-7 batch 1. Within a batch group,
core q owns d_inner channels [384q, 384q+384). Activations live in
[feature, time] (transposed) layout so the depthwise conv and the selective
scan run along the free dimension. The scan uses the VectorE
tensor_tensor_scan instruction over a [(channel,state), time] layout; the
16x state broadcast of dt/u is done with selector matmuls on TensorE, and
the y readout (sum over state) with selector matmuls accumulating in PSUM.
x_proj and out_proj contract over the full d_inner, so their partials are
AllReduced within each batch group. The LM head is sharded 4-way over vocab
within each batch group (each core computes [8000, 2048] logits^T).
"""
import numpy as np
import ml_dtypes

import concourse.bass as bass
import concourse.tile as tile
from concourse import mybir
from concourse.bass_utils import run_bass_kernel_spmd
from concourse.vector_clock import ScopedClock

# ---------------------------------------------------------------- tile fix --
# This walrus build rejects >1 sem-wait on some instruction encodings (CTRL
# drain/nop, S2S2D2 scan). After Tile scheduling, spill excess waits onto
# same-engine nops inserted right before the overloaded instruction.
MAX_WAITS = 1


def _redistribute_waits(nc, cap=MAX_WAITS):
    work = []
    for f in nc.m.functions:
        for bb in f.blocks:
            insts = bb.instructions
            ov = set()
            for i, inst in enumerate(insts):
                si = inst.sync_info
                if si is not None and si.on_wait and len(si.on_wait) > cap:
                    ov.add(i)
            if ov:
                work.append((bb, insts, ov))
    if not work:
        return
    curbb = nc.cur_bb.bb

    def make_nop(engine, waits):
        eng = nc.engines[engine]
        nop = eng.nop(nofuse=True).ins
        lst = curbb.instructions
        assert lst[-1].name == nop.name, "nop did not append to cur_bb tail"
        curbb.instructions = lst[:-1]
        nop.sync_info = mybir.SyncInfo(on_wait=list(waits), on_update=[])
        return nop

    for bb, insts, ov in work:
        newlist = []
        for i, inst in enumerate(insts):
            if i in ov:
                si = inst.sync_info
                waits = list(si.on_wait)
                si.on_wait = waits[:cap]
                rest = waits[cap:]
                while rest:
                    chunk, rest = rest[:cap], rest[cap:]
                    newlist.append(make_nop(inst.engine, chunk))
            newlist.append(inst)
        bb.instructions = newlist


class _TileCtx(tile.TileContext):
    def _drain_and_barrier(self, tick_clock, wait_clock):
        nc = self.nc
        drain_inst = nc.sync.drain()
        wait_clock.add_sem_waits(
            drain_inst.ins, ScopedClock({None: tick_clock.global_clock})
        )
        si = drain_inst.ins.sync_info
        if si is not None and si.on_wait and len(si.on_wait) > MAX_WAITS:
            waits = list(si.on_wait)
            si.on_wait = waits[:MAX_WAITS]
            rest = waits[MAX_WAITS:]
            while rest:
                extra = nc.sync.drain()
                chunk, rest = rest[:MAX_WAITS], rest[MAX_WAITS:]
                esi = extra.ins.sync_info
                if esi is None:
                    extra.ins.sync_info = mybir.SyncInfo(
                        on_wait=list(chunk), on_update=[]
                    )
                else:
                    esi.on_wait = list(chunk)
        nc.all_engine_barrier()
        assert self.sems is not None
        popped = nc._tile_sem_poison_stack.pop()
        assert popped is self._sem_poison
        nc.clear_and_free_semaphores(list(self.sems.allocated().values()))
        nc.all_engine_barrier()

    def __exit__(self, exc_type, exc, tb):
        ret = super().__exit__(exc_type, exc, tb)
        if exc_type is None:
            _redistribute_waits(self.nc)
        return ret


# ---------------------------------------------------------------- constants --
VOCAB, DIM, DEPTH = 32000, 768, 4
DS, DCONV, DIN, DTR = 16, 4, 1536, 48
B, T = 2, 2048
NCORES = 8
DSH = DIN // 4          # 384 channels per core
VSH = VOCAB // 4        # 8000 vocab rows per core (within its batch group)
NCT = DSH // 128        # 3 channel tiles
NKT = DIM // 128        # 6 dim tiles
NJ = T // 512           # 4 time chunks of 512
NG = DSH * DS // 128    # 48 (c,s) tiles
EPS = 1e-5

FP32 = mybir.dt.float32
BF16 = mybir.dt.bfloat16
AF = mybir.ActivationFunctionType
OP = mybir.AluOpType
BF = ml_dtypes.bfloat16

REPLICA_GROUPS = [[0, 1, 2, 3], [4, 5, 6, 7]]


def _build_program():
    nc = bass.Bass()

    # ---- I/O -------------------------------------------------------------
    d_xT = nc.dram_tensor("xT", [DIM, T], BF16, kind="ExternalInput")
    d_w_in = nc.dram_tensor("w_in_T", [DEPTH, DIM, 2 * DSH], BF16, kind="ExternalInput")
    d_conv_w = nc.dram_tensor("conv_wk", [DEPTH, DCONV, NCT, 128], FP32, kind="ExternalInput")
    d_conv_b = nc.dram_tensor("conv_b", [DEPTH, NCT, 128], FP32, kind="ExternalInput")
    d_w_x = nc.dram_tensor("w_x_T", [DEPTH, DSH, 80], BF16, kind="ExternalInput")
    d_w_dt = nc.dram_tensor("w_dt_T", [DEPTH, DTR, DSH], BF16, kind="ExternalInput")
    d_b_dt = nc.dram_tensor("b_dt", [DEPTH, NCT, 128], FP32, kind="ExternalInput")
    d_a_cs = nc.dram_tensor("a_cs", [DEPTH, NG, 128], FP32, kind="ExternalInput")
    d_dp = nc.dram_tensor("dp", [DEPTH, NCT, 128], FP32, kind="ExternalInput")
    d_w_out = nc.dram_tensor("w_out_T", [DEPTH, DSH, DIM], BF16, kind="ExternalInput")
    d_ln_g = nc.dram_tensor("ln_g", [NKT, 128], FP32, kind="ExternalInput")
    d_ln_b = nc.dram_tensor("ln_b", [NKT, 128], FP32, kind="ExternalInput")
    d_w_head = nc.dram_tensor("w_head_T", [DIM, VSH], BF16, kind="ExternalInput")
    d_sel8g = nc.dram_tensor("sel8g", [16, 128, 128], BF16, kind="ExternalInput")
    d_selg = nc.dram_tensor("selg", [16, 128, 128], BF16, kind="ExternalInput")
    d_sel16bc = nc.dram_tensor("sel16bc", [16, 128], BF16, kind="ExternalInput")
    d_ones_row = nc.dram_tensor("ones_row", [1, 128], FP32, kind="ExternalInput")
    d_ones_col = nc.dram_tensor("ones_col", [128, 1], FP32, kind="ExternalInput")
    d_ones_colb = nc.dram_tensor("ones_colb", [128, 1], BF16, kind="ExternalInput")

    d_logT = nc.dram_tensor("logitsT", [VSH, T], FP32, kind="ExternalOutput")

    with _TileCtx(nc) as tc:
        with (
            tc.tile_pool(name="const", bufs=1) as cpool,
            tc.tile_pool(name="xcur", bufs=1) as xpool,
            tc.tile_pool(name="wts", bufs=1) as wpool,
            tc.tile_pool(name="whp", bufs=2) as whpool,
            tc.tile_pool(name="act", bufs=1) as apool,
            tc.tile_pool(name="big", bufs=5) as bpool,
            tc.tile_pool(name="scan", bufs=2) as spool,
            tc.tile_pool(name="tmp", bufs=2) as tpool,
            tc.tile_pool(name="ln", bufs=1) as lpool,
            tc.tile_pool(name="ps", bufs=2, space="PSUM") as pspool,
            tc.tile_pool(name="psy", bufs=1, space="PSUM") as pypool,
            tc.tile_pool(name="dram", bufs=2, space="DRAM") as dpool,
        ):
            # ---- constants ---------------------------------------------
            sel8g = cpool.tile([128, 16 * 128], BF16, tag="sel8g")
            selg = cpool.tile([128, 16 * 128], BF16, tag="selg")
            sel16bc = cpool.tile([16, 128], BF16, tag="sel16bc")
            ones_row = cpool.tile([1, 128], FP32, tag="ones_row")
            ones_col = cpool.tile([128, 1], FP32, tag="ones_col")
            ones_colb = cpool.tile([128, 1], BF16, tag="ones_colb")
            for gg in range(16):
                nc.sync.dma_start(sel8g[:, 128 * gg:128 * (gg + 1)], d_sel8g[gg])
                nc.sync.dma_start(selg[:, 128 * gg:128 * (gg + 1)], d_selg[gg])
            nc.sync.dma_start(sel16bc[:], d_sel16bc[:])
            nc.sync.dma_start(ones_row[:], d_ones_row[:])
            nc.sync.dma_start(ones_col[:], d_ones_col[:])
            nc.sync.dma_start(ones_colb[:], d_ones_colb[:])
            lng = cpool.tile([128, NKT], FP32, tag="lng")
            lnb = cpool.tile([128, NKT], FP32, tag="lnb")
            nc.sync.dma_start(lng[:], d_ln_g.rearrange("k p -> p k"))
            nc.sync.dma_start(lnb[:], d_ln_b.rearrange("k p -> p k"))

            # ---- x current (bf16, [128, T] x 6) ------------------------
            xc = []
            for k in range(NKT):
                t_ = xpool.tile([128, T], BF16, tag=f"xc{k}", name=f"xc{k}")
                nc.sync.dma_start(t_[:], d_xT[128 * k:128 * (k + 1), :])
                xc.append(t_)

            for l in range(DEPTH):
                # ---- load layer weights --------------------------------
                w_in = []
                for k in range(NKT):
                    t_ = wpool.tile([128, 2 * DSH], BF16, tag=f"win{k}",
                                    name=f"win{k}")
                    nc.sync.dma_start(t_[:], d_w_in[l, 128 * k:128 * (k + 1), :])
                    w_in.append(t_)
                w_x = []
                for c in range(NCT):
                    t_ = wpool.tile([128, 80], BF16, tag=f"wx{c}", name=f"wx{c}")
                    nc.sync.dma_start(t_[:], d_w_x[l, 128 * c:128 * (c + 1), :])
                    w_x.append(t_)
                w_dt = wpool.tile([DTR, DSH], BF16, tag="wdt", name="wdt")
                nc.sync.dma_start(w_dt[:], d_w_dt[l])
                w_out = []
                for c in range(NCT):
                    t_ = wpool.tile([128, DIM], BF16, tag=f"wo{c}", name=f"wo{c}")
                    nc.sync.dma_start(t_[:], d_w_out[l, 128 * c:128 * (c + 1), :])
                    w_out.append(t_)
                convw = wpool.tile([128, DCONV * NCT], FP32, tag="convw",
                                   name="convw")
                nc.sync.dma_start(convw[:], d_conv_w[l].rearrange("k c p -> p (k c)"))
                convb = wpool.tile([128, NCT], FP32, tag="convb", name="convb")
                nc.sync.dma_start(convb[:], d_conv_b[l].rearrange("c p -> p c"))
                bdt = wpool.tile([128, NCT], FP32, tag="bdt", name="bdt")
                nc.sync.dma_start(bdt[:], d_b_dt[l].rearrange("c p -> p c"))
                acs = wpool.tile([128, NG], FP32, tag="acs", name="acs")
                nc.sync.dma_start(acs[:], d_a_cs[l].rearrange("g p -> p g"))
                dpw = wpool.tile([128, NCT], FP32, tag="dpw", name="dpw")
                nc.sync.dma_start(dpw[:], d_dp[l].rearrange("c p -> p c"))

                # ---- in_proj: xz^T [768out, T]; x_in has 4-col halo ----
                _sc_inproj = nc.named_scope(f"L{l}_inproj"); _sc_inproj.__enter__()
                x_in = []
                z_silu = []
                for m in range(6):
                    if m < 3:
                        xt = bpool.tile([128, T + 4], BF16, tag="big",
                                        name=f"xin{m}")
                        nc.vector.memset(xt[:, 0:4], 0.0)
                        x_in.append(xt)
                    else:
                        zt = apool.tile([128, T], BF16, tag=f"zs{m - 3}",
                                        name=f"zs{m - 3}")
                        z_silu.append(zt)
                    for j in range(NJ):
                        ps = pspool.tile([128, 512], FP32, tag="ps_a", name="psa")
                        for k in range(NKT):
                            nc.tensor.matmul(
                                ps[:], w_in[k][:, 128 * m:128 * (m + 1)],
                                xc[k][:, 512 * j:512 * (j + 1)],
                                start=(k == 0), stop=(k == NKT - 1))
                        if m < 3:
                            nc.scalar.copy(
                                x_in[m][:, 4 + 512 * j:4 + 512 * (j + 1)], ps[:])
                        else:
                            nc.scalar.activation(
                                z_silu[m - 3][:, 512 * j:512 * (j + 1)], ps[:],
                                AF.Silu)

                _sc_inproj.__exit__(None, None, None)
                _sc_conv = nc.named_scope(f"L{l}_conv"); _sc_conv.__enter__()
                # ---- conv + silu -> x_ssm ------------------------------
                x_ssm = []
                for c in range(NCT):
                    cv = tpool.tile([128, T], BF16, tag="cv", name="cv")
                    nc.vector.tensor_scalar(
                        cv[:], x_in[c][:, 1:1 + T],
                        convw[:, 0 * NCT + c:0 * NCT + c + 1], None, OP.mult)
                    for k in range(1, DCONV):
                        cv2 = tpool.tile([128, T], BF16, tag="cv", name="cv")
                        nc.vector.scalar_tensor_tensor(
                            cv2[:], x_in[c][:, 1 + k:1 + k + T],
                            convw[:, k * NCT + c:k * NCT + c + 1], cv[:],
                            OP.mult, OP.add)
                        cv = cv2
                    xs = apool.tile([128, T], BF16, tag=f"xssm{c}",
                                    name=f"xssm{c}")
                    nc.scalar.activation(xs[:], cv[:], AF.Silu,
                                         bias=convb[:, c:c + 1])
                    x_ssm.append(xs)

                _sc_conv.__exit__(None, None, None)
                _sc_xproj = nc.named_scope(f"L{l}_xproj"); _sc_xproj.__enter__()
                # ---- x_proj (partial) + AllReduce ----------------------
                dblp = tpool.tile([80, T], BF16, tag="dblp", name="dblp",
                                  bufs=1)
                for j in range(NJ):
                    ps = pspool.tile([80, 512], FP32, tag="ps_a", name="psa")
                    for c in range(NCT):
                        nc.tensor.matmul(
                            ps[:], w_x[c][:],
                            x_ssm[c][:, 512 * j:512 * (j + 1)],
                            start=(c == 0), stop=(c == NCT - 1))
                    nc.scalar.copy(dblp[:, 512 * j:512 * (j + 1)], ps[:])
                d_dbl_p = dpool.tile([80, T], FP32, tag="dbl_p", name="dbl_p")
                d_dbl_f = dpool.tile([80, T], FP32, tag="dbl_f", name="dbl_f")
                nc.gpsimd.dma_start(d_dbl_p[:], dblp[:])
                nc.gpsimd.collective_compute(
                    "AllReduce", OP.add, replica_groups=REPLICA_GROUPS,
                    ins=[d_dbl_p.opt()], outs=[d_dbl_f.opt()])
                # split into base-0 tiles (matmul operands must share base 0)
                dbl_dt = apool.tile([DTR, T], BF16, tag="dbl_dt", name="dbl_dt")
                dbl_B = apool.tile([DS, T], BF16, tag="dbl_B", name="dbl_B")
                dbl_C = apool.tile([DS, T], BF16, tag="dbl_C", name="dbl_C")
                nc.gpsimd.dma_start(dbl_dt[:], d_dbl_f[0:DTR, :])
                nc.gpsimd.dma_start(dbl_B[:], d_dbl_f[DTR:DTR + DS, :])
                nc.gpsimd.dma_start(dbl_C[:], d_dbl_f[DTR + DS:80, :])

                _sc_xproj.__exit__(None, None, None)
                _sc_bc = nc.named_scope(f"L{l}_bc"); _sc_bc.__enter__()
                # ---- B_bc / C_bc [128, T] (bf16) -----------------------
                Bbc = apool.tile([128, T], BF16, tag="Bbc", name="Bbc")
                Cbc = apool.tile([128, T], BF16, tag="Cbc", name="Cbc")
                for j in range(NJ):
                    psb = pspool.tile([128, 512], FP32, tag="ps_a", name="psa")
                    nc.tensor.matmul(psb[:], sel16bc[:],
                                     dbl_B[:, 512 * j:512 * (j + 1)],
                                     start=True, stop=True)
                    nc.scalar.copy(Bbc[:, 512 * j:512 * (j + 1)], psb[:])
                    psc = pspool.tile([128, 512], FP32, tag="ps_a", name="psa")
                    nc.tensor.matmul(psc[:], sel16bc[:],
                                     dbl_C[:, 512 * j:512 * (j + 1)],
                                     start=True, stop=True)
                    nc.scalar.copy(Cbc[:, 512 * j:512 * (j + 1)], psc[:])

                _sc_bc.__exit__(None, None, None)
                _sc_scan = nc.named_scope(f"L{l}_scan"); _sc_scan.__enter__()
                # ---- scan (dt/u computed per channel-tile) -------------
                ydp = []
                for c in range(NCT):
                    dt_ = apool.tile([128, T], BF16, tag="dt", name="dt")
                    for j in range(NJ):
                        ps = pspool.tile([128, 512], FP32, tag="ps_a", name="psa")
                        nc.tensor.matmul(
                            ps[:], w_dt[:, 128 * c:128 * (c + 1)],
                            dbl_dt[:, 512 * j:512 * (j + 1)],
                            start=True, stop=True)
                        # softplus(x) = ln(1 + exp(x)); Softplus has no ACT table
                        spe = tpool.tile([128, 512], FP32, tag="spe", name="spe")
                        nc.scalar.activation(spe[:], ps[:], AF.Exp,
                                             bias=bdt[:, c:c + 1])
                        sp1 = tpool.tile([128, 512], FP32, tag="sp1", name="sp1")
                        nc.vector.tensor_scalar(sp1[:], spe[:], 1.0, None, OP.add)
                        nc.scalar.activation(
                            dt_[:, 512 * j:512 * (j + 1)], sp1[:], AF.Ln)
                    u_ = apool.tile([128, T], BF16, tag="u", name="u")
                    nc.vector.tensor_mul(u_[:], dt_[:], x_ssm[c][:])

                    yps = [pypool.tile([128, 512], FP32, tag=f"yps{j}",
                                       name=f"yps{j}") for j in range(NJ)]
                    for gg in range(16):
                        g = 16 * c + gg
                        dA = spool.tile([128, T], BF16, tag="scan_a", name="dA")
                        for j in range(NJ):
                            ps = pspool.tile([128, 512], FP32, tag="ps_c",
                                             name="psc")
                            nc.tensor.matmul(
                                ps[:], sel8g[:, 128 * gg:128 * (gg + 1)],
                                dt_[:, 512 * j:512 * (j + 1)],
                                start=True, stop=True)
                            nc.scalar.activation(
                                dA[:, 512 * j:512 * (j + 1)], ps[:], AF.Exp,
                                scale=acs[:, g:g + 1])
                        uB = spool.tile([128, T], BF16, tag="scan_b", name="uB")
                        for j in range(NJ):
                            ps = pspool.tile([128, 512], FP32, tag="ps_c",
                                             name="psc")
                            nc.tensor.matmul(
                                ps[:], sel8g[:, 128 * gg:128 * (gg + 1)],
                                u_[:, 512 * j:512 * (j + 1)],
                                start=True, stop=True)
                            nc.vector.tensor_mul(
                                uB[:, 512 * j:512 * (j + 1)], ps[:],
                                Bbc[:, 512 * j:512 * (j + 1)])
                        h = spool.tile([128, T], BF16, tag="scan_a", name="h")
                        nc.vector.tensor_tensor_scan(
                            h[:], dA[:], uB[:], 0.0, OP.mult, OP.add)
                        hC = spool.tile([128, T], BF16, tag="scan_b", name="hC")
                        nc.gpsimd.tensor_mul(hC[:], h[:], Cbc[:])
                        for j in range(NJ):
                            nc.tensor.matmul(
                                yps[j][:], selg[:, 128 * gg:128 * (gg + 1)],
                                hC[:, 512 * j:512 * (j + 1)],
                                start=(gg == 0), stop=(gg == 15))
                    # y + x_ssm*Dp directly from PSUM
                    yd = bpool.tile([128, T], BF16, tag="big", name=f"ydp{c}")
                    for j in range(NJ):
                        nc.vector.scalar_tensor_tensor(
                            yd[:, 512 * j:512 * (j + 1)],
                            x_ssm[c][:, 512 * j:512 * (j + 1)],
                            dpw[:, c:c + 1], yps[j][:], OP.mult, OP.add)
                    ydp.append(yd)

                _sc_scan.__exit__(None, None, None)
                _sc_out = nc.named_scope(f"L{l}_outproj"); _sc_out.__enter__()
                # ---- gate + out_proj -----------------------------------
                yg = []
                for c in range(NCT):
                    t2 = bpool.tile([128, T], BF16, tag="big", name=f"yg{c}")
                    nc.vector.tensor_mul(t2[:], ydp[c][:], z_silu[c][:])
                    yg.append(t2)

                d_xn_p = dpool.tile([DIM, T], FP32, tag="xn_p", name="xn_p")
                d_xn_f = dpool.tile([DIM, T], FP32, tag="xn_f", name="xn_f")
                for m in range(NKT):
                    xnp = tpool.tile([128, T], FP32, tag="xnp", name="xnp")
                    for j in range(NJ):
                        ps = pspool.tile([128, 512], FP32, tag="ps_a", name="psa")
                        for c in range(NCT):
                            nc.tensor.matmul(
                                ps[:], w_out[c][:, 128 * m:128 * (m + 1)],
                                yg[c][:, 512 * j:512 * (j + 1)],
                                start=(c == 0), stop=(c == NCT - 1))
                        nc.scalar.copy(xnp[:, 512 * j:512 * (j + 1)], ps[:])
                    nc.sync.dma_start(d_xn_p[128 * m:128 * (m + 1), :], xnp[:])
                    if m == 2:
                        nc.gpsimd.collective_compute(
                            "AllReduce", OP.add, replica_groups=REPLICA_GROUPS,
                            ins=[d_xn_p[0:384, :]], outs=[d_xn_f[0:384, :]])
                nc.gpsimd.collective_compute(
                    "AllReduce", OP.add, replica_groups=REPLICA_GROUPS,
                    ins=[d_xn_p[384:DIM, :]], outs=[d_xn_f[384:DIM, :]])
                for k in range(NKT):
                    nc.gpsimd.dma_start(xc[k][:], d_xn_f[128 * k:128 * (k + 1), :])
                _sc_out.__exit__(None, None, None)

            # ---- layernorm (per 512-chunk, in-place on xc) -------------
            _sc_ln = nc.named_scope("LN"); _sc_ln.__enter__()
            for j in range(NJ):
                js = slice(512 * j, 512 * (j + 1))
                ps1 = pspool.tile([1, 512], FP32, tag="ps_a", name="psa")
                for k in range(NKT):
                    nc.tensor.matmul(ps1[:], ones_colb[:], xc[k][:, js],
                                     start=(k == 0), stop=(k == NKT - 1))
                ps2 = pspool.tile([1, 512], FP32, tag="ps_a", name="psa")
                for k in range(NKT):
                    sqt = tpool.tile([128, 512], FP32, tag="sq", name="sq")
                    nc.scalar.activation(sqt[:], xc[k][:, js], AF.Square)
                    nc.tensor.matmul(ps2[:], ones_col[:], sqt[:],
                                     start=(k == 0), stop=(k == NKT - 1))
                mu = lpool.tile([1, 512], FP32, tag="mu", name="mu")
                nc.scalar.mul(mu[:], ps1[:], 1.0 / DIM)
                musq = lpool.tile([1, 512], FP32, tag="musq", name="musq")
                nc.scalar.activation(musq[:], mu[:], AF.Square)
                s2n = lpool.tile([1, 512], FP32, tag="s2n", name="s2n")
                nc.scalar.mul(s2n[:], ps2[:], 1.0 / DIM)
                var = lpool.tile([1, 512], FP32, tag="var", name="var")
                nc.vector.tensor_sub(var[:], s2n[:], musq[:])
                vare = lpool.tile([1, 512], FP32, tag="vare", name="vare")
                nc.vector.tensor_scalar(vare[:], var[:], float(EPS), None, OP.add)
                std = lpool.tile([1, 512], FP32, tag="std", name="std")
                nc.scalar.activation(std[:], vare[:], AF.Sqrt)
                rstd = lpool.tile([1, 512], FP32, tag="rstd", name="rstd")
                nc.vector.reciprocal(rstd[:], std[:])
                # broadcast to 128 partitions
                psm = pspool.tile([128, 512], FP32, tag="ps_a", name="psa")
                nc.tensor.matmul(psm[:], ones_row[:], mu[:], start=True, stop=True)
                mub = lpool.tile([128, 512], BF16, tag="mub", name="mub")
                nc.scalar.copy(mub[:], psm[:])
                psr = pspool.tile([128, 512], FP32, tag="ps_a", name="psa")
                nc.tensor.matmul(psr[:], ones_row[:], rstd[:], start=True,
                                 stop=True)
                rsb = lpool.tile([128, 512], BF16, tag="rsb", name="rsb")
                nc.scalar.copy(rsb[:], psr[:])
                for k in range(NKT):
                    d1 = lpool.tile([128, 512], BF16, tag="lnd1", name="lnd1")
                    nc.vector.tensor_sub(d1[:], xc[k][:, js], mub[:])
                    d2 = lpool.tile([128, 512], BF16, tag="lnd2", name="lnd2")
                    nc.vector.tensor_mul(d2[:], d1[:], rsb[:])
                    nc.scalar.activation(xc[k][:, js], d2[:], AF.Identity,
                                         bias=lnb[:, k:k + 1],
                                         scale=lng[:, k:k + 1])

            _sc_ln.__exit__(None, None, None)
            _sc_head = nc.named_scope("head"); _sc_head.__enter__()
            # ---- head: logits^T [8000, 2048] ---------------------------
            vt_sizes = [128] * (VSH // 128) + ([VSH % 128] if VSH % 128 else [])
            for vi, vsz in enumerate(vt_sizes):
                wh = whpool.tile([128, NKT * 128], BF16, tag="wh", name="wh")
                for k in range(NKT):
                    nc.sync.dma_start(
                        wh[:, 128 * k:128 * k + vsz],
                        d_w_head[128 * k:128 * (k + 1), 128 * vi:128 * vi + vsz])
                hps = [pypool.tile([128, 512], FP32, tag=f"yps{j}",
                                   name=f"hps{j}") for j in range(NJ)]
                for k in range(NKT):
                    for j in range(NJ):
                        nc.tensor.matmul(
                            hps[j][0:vsz, :], wh[:, 128 * k:128 * k + vsz],
                            xc[k][:, 512 * j:512 * (j + 1)],
                            start=(k == 0), stop=(k == NKT - 1))
                for j in range(NJ):
                    lt = tpool.tile([128, 512], FP32, tag="lt", name="lt",
                                    bufs=2)
                    nc.scalar.copy(lt[0:vsz, :], hps[j][0:vsz, :])
                    nc.sync.dma_start(
                        d_logT[128 * vi:128 * vi + vsz, 512 * j:512 * (j + 1)],
                        lt[0:vsz, :])
            _sc_head.__exit__(None, None, None)
    return nc


# ------------------------------------------------------------------ host ----
_PROG = None


def _get_program():
    global _PROG
    if _PROG is None:
        _PROG = _build_program()
    return _PROG


def _selectors():
    p_ar = np.arange(128)
    sel8g = np.zeros((16, 128, 128), BF)
    selg = np.zeros((16, 128, 128), BF)
    for gg in range(16):
        sel8g[gg, 8 * gg + p_ar // 16, p_ar] = 1.0
        selg[gg, p_ar, 8 * gg + p_ar // 16] = 1.0
    sel16bc = np.zeros((16, 128), BF)
    sel16bc[p_ar % 16, p_ar] = 1.0
    return sel8g, selg, sel16bc


def _prep_core_inputs(core, ids, emb, W_in, conv_w, conv_b, W_x, W_dt, b_dt,
                      A_log, Dp, W_out, ln_g, ln_b, W_head, sels):
    b = core // 4
    q = core % 4
    csl = slice(q * DSH, (q + 1) * DSH)
    vsl = slice(q * VSH, (q + 1) * VSH)
    sel8g, selg, sel16bc = sels

    xT = np.ascontiguousarray(emb[ids[b]].T).astype(BF)   # [768, 2048]

    w_in_T = np.empty((DEPTH, DIM, 2 * DSH), BF)
    w_x_T = np.empty((DEPTH, DSH, 80), BF)
    w_dt_T = np.empty((DEPTH, DTR, DSH), BF)
    w_out_T = np.empty((DEPTH, DSH, DIM), BF)
    conv_wk = np.empty((DEPTH, DCONV, NCT, 128), np.float32)
    conv_b_r = np.empty((DEPTH, NCT, 128), np.float32)
    b_dt_r = np.empty((DEPTH, NCT, 128), np.float32)
    dp_r = np.empty((DEPTH, NCT, 128), np.float32)
    a_cs = np.empty((DEPTH, NG, 128), np.float32)
    p_ar = np.arange(128)
    for l in range(DEPTH):
        rows = np.concatenate([np.arange(csl.start, csl.stop),
                               DIN + np.arange(csl.start, csl.stop)])
        w_in_T[l] = W_in[l][rows, :].T.astype(BF)
        w_x_T[l] = W_x[l][:, csl].T.astype(BF)
        w_dt_T[l] = W_dt[l][csl, :].T.astype(BF)
        w_out_T[l] = W_out[l][:, csl].T.astype(BF)
        conv_wk[l] = conv_w[l][csl, :].T.reshape(DCONV, NCT, 128)
        conv_b_r[l] = conv_b[l][csl].reshape(NCT, 128)
        b_dt_r[l] = b_dt[l][csl].reshape(NCT, 128)
        dp_r[l] = Dp[l][csl].reshape(NCT, 128)
        A = -np.exp(A_log[l][csl, :])                     # [384, 16]
        for g in range(NG):
            a_cs[l, g] = A[8 * g + p_ar // 16, p_ar % 16]

    return {
        "xT": xT,
        "w_in_T": w_in_T, "conv_wk": conv_wk, "conv_b": conv_b_r,
        "w_x_T": w_x_T, "w_dt_T": w_dt_T, "b_dt": b_dt_r,
        "a_cs": a_cs, "dp": dp_r, "w_out_T": w_out_T,
        "ln_g": np.ascontiguousarray(ln_g.reshape(NKT, 128)).astype(np.float32),
        "ln_b": np.ascontiguousarray(ln_b.reshape(NKT, 128)).astype(np.float32),
        "w_head_T": np.ascontiguousarray(W_head[vsl, :].T).astype(BF),
        "sel8g": sel8g, "selg": selg, "sel16bc": sel16bc,
        "ones_row": np.ones((1, 128), np.float32),
        "ones_col": np.ones((128, 1), np.float32),
        "ones_colb": np.ones((128, 1), BF),
    }


def kernel(input_ids, emb, W_in, conv_w, conv_b, W_x, W_dt, b_dt, A_log, Dp,
           W_out, ln_g, ln_b, W_head):
    ids = np.asarray(input_ids).astype(np.int64)
    args = [np.asarray(a, np.float32) for a in
            (emb, W_in, conv_w, conv_b, W_x, W_dt, b_dt, A_log, Dp, W_out,
             ln_g, ln_b, W_head)]
    nc = _get_program()
    sels = _selectors()
    in_maps = [_prep_core_inputs(c, ids, *args, sels) for c in range(NCORES)]
    res = run_bass_kernel_spmd(nc, in_maps, list(range(NCORES))).results
    out = np.empty((B, T, VOCAB), np.float32)
    for b in range(B):
        for q in range(4):
            out[b, :, q * VSH:(q + 1) * VSH] = res[4 * b + q]["logitsT"].T
    return out
